# revision 10
# baseline (speedup 1.0000x reference)
"""BiLSTM-CRF loss kernel for 8x Trainium2 NeuronCores (Bass/Tile).

Sharding: data-parallel over batch (16 sentences per core). Each core runs the
identical SPMD program: embedding gather -> 2 BiLSTM layers (fwd+bwd scans
interleaved per tick) -> emissions -> bidirectional CRF partition function +
path-score numerator. Host sums per-core partials and adds the tags-only part
of the numerator.

Design notes (v3, latency-oriented):
 - The per-tick recurrence is latency-bound (engines are mostly idle), so the
   cell is laid out feature-on-partition: z/s/c/h are [128, batch] tiles.
   Act/DVE instruction cost = free-dim elems + fixed access-latency init, so
   narrow free dims win, and h is produced in exactly the [H, B] layout the
   next tick's matmul consumes (no transpose, no history copy).
 - Gate preactivations per direction: psum tile [128, 4*BL]; per gate block a
   rank-1 bias matmul + x-proj + h-proj accumulate in bf16 (1 PE cycle/row).
 - tanh(x) = 2*sigmoid(2x) - 1 everywhere, so one Sigmoid activation covers
   all four gates.  With h~ = h/2 and c~ = c/2 (factors of 2 folded into the
   weights host-side):
     s = sigmoid(z'), z' row-scaled so s_g = sigmoid(2 z_g)
     u  = (s_g - 0.5) * s_i          ( = i*g/2 )
     c~ = s_f * c~_prev + u
     h~ = (sigmoid(4 c~) - 0.5) * s_o
 - Per-tick critical cycle: h-matmul -> sigmoid[128,64] -> u/fc/cn (DVE) ->
   sigmoid(4c)[128,16] -> h-write (DVE, straight into the history buffer).
   fwd and bwd run as two independent chains that fill each other's idle.
 - Emissions are computed after the scans, outside-in (chunks 0 and N-1
   first) so the CRF can start immediately and the remaining emission chunks
   hide under its latency-bound chains.
 - CRF partition function is bidirectional to halve the sequential chain:
   alpha runs t=0..M-1 (exp space, a_t = e_t .* (Eexp^T a_{t-1})), beta runs
   t=T-1..M (b_t = e_t .* (Eexp b_{t+1}), includes end), concurrently; then
   Z = (Eexp^T a_{M-1}) . b_M.  Partition-sum rescale every RESCALE steps
   (applied two steps deferred, off the critical chain), log accumulated.
 - Numerator: device computes only sum_t em[y_t, t] per sentence (one
   accumulating STT against host-marshalled one-hot tags, interleaved into
   the CRF); the tags-only part (start/end/transition sums) is computed on
   the host directly from the inputs.
"""

import sys

sys.path.insert(0, "/opt/trn_rl_repo")

import contextlib

import numpy as np
import ml_dtypes

import concourse.bass as bass
import concourse.tile as tile
from concourse import bacc, mybir
from concourse.bass_utils import run_bass_kernel_spmd

F32 = mybir.dt.float32
F32R = mybir.dt.float32r
BF16 = mybir.dt.bfloat16
I16 = mybir.dt.int16
AF = mybir.ActivationFunctionType
OP = mybir.AluOpType

NCORES = 8
B, T, E, H, K, V = 128, 512, 128, 128, 20, 30000
G4 = 4 * H          # 512
BL = B // NCORES    # 16 sentences per core
RESCALE = 8


def _mm(nc, out, lhsT, rhs, start, stop):
    nc.tensor.matmul(out, lhsT, rhs, start=start, stop=stop)


def build(nt=T):
    """Build the SPMD program for sequence length nt (nt=T for real use)."""
    nc = bacc.Bacc("TRN2", target_bir_lowering=False, debug=False,
                   num_devices=NCORES)
    NTB = nt * BL   # flattened (t,b) count per core

    # ---- DRAM I/O ----
    embedb = nc.dram_tensor("embedb", [V, E], BF16, kind="ExternalInput")
    toks16 = nc.dram_tensor("toks16", [BL, nt], I16, kind="ExternalInput")
    tagoh = nc.dram_tensor("tagoh", [K, NTB], BF16, kind="ExternalInput")  # one-hot, b-major
    wihT0 = nc.dram_tensor("wihT0", [2, E, G4], BF16, kind="ExternalInput")
    whhT0 = nc.dram_tensor("whhT0", [2, H, G4], BF16, kind="ExternalInput")
    b0v = nc.dram_tensor("b0v", [2, 1, G4], BF16, kind="ExternalInput")
    wih1T = nc.dram_tensor("wih1T", [2, 2, H, G4], BF16, kind="ExternalInput")
    whh1T = nc.dram_tensor("whh1T", [2, H, G4], BF16, kind="ExternalInput")
    b1v = nc.dram_tensor("b1v", [2, 1, G4], BF16, kind="ExternalInput")
    woutT = nc.dram_tensor("woutT", [2, H, K], BF16, kind="ExternalInput")
    boutv = nc.dram_tensor("boutv", [K, 1], F32, kind="ExternalInput")
    transm = nc.dram_tensor("transm", [K, K], F32, kind="ExternalInput")
    transmT = nc.dram_tensor("transmT", [K, K], F32, kind="ExternalInput")
    startv = nc.dram_tensor("startv", [K, 1], F32, kind="ExternalInput")
    endv = nc.dram_tensor("endv", [K, 1], F32, kind="ExternalInput")
    outm = nc.dram_tensor("outm", [2, BL], F32, kind="ExternalOutput")

    with tile.TileContext(nc) as tc, contextlib.ExitStack() as ctx:
        big = ctx.enter_context(tc.tile_pool(name="big", bufs=1))
        wp = ctx.enter_context(tc.tile_pool(name="wp", bufs=1))
        work = ctx.enter_context(tc.tile_pool(name="work", bufs=3))
        stp = ctx.enter_context(tc.tile_pool(name="stp", bufs=2))

        # ---------------- P0: constants, weights, gather ----------------
        idx = wp.tile([128, nt], I16, tag="idx")
        nc.gpsimd.memset(idx[:], 0)
        nc.sync.dma_start(out=idx[0:BL, :], in_=toks16[:, :])

        def load_w(name, dram_ap, shape, dt=F32):
            t = wp.tile(shape, dt, tag=name)
            nc.sync.dma_start(out=t[:], in_=dram_ap)
            return t

        wih0_sb = [load_w(f"wih0_{d}", wihT0[d], [E, G4], BF16)
                   for d in range(2)]
        whh0_sb = [load_w(f"whh0_{d}", whhT0[d], [H, G4], BF16)
                   for d in range(2)]
        b0_sb = [load_w(f"b0_{d}", b0v[d], [1, G4], BF16) for d in range(2)]
        wih1_sb = [[load_w(f"wih1_{d}{h}", wih1T[d, h], [H, G4], BF16)
                    for h in range(2)] for d in range(2)]
        whh1_sb = [load_w(f"whh1_{d}", whh1T[d], [H, G4], BF16)
                   for d in range(2)]
        b1_sb = [load_w(f"b1_{d}", b1v[d], [1, G4], BF16) for d in range(2)]
        wout_sb = [load_w(f"wout_{d}", woutT[d], [H, K], BF16)
                   for d in range(2)]
        bout_sb = load_w("bout", boutv[:, :], [K, 1])
        trans_sb = load_w("trans", transm[:, :], [K, K])
        transT_sb = load_w("transT", transmT[:, :], [K, K])
        start_sb = load_w("start", startv[:, :], [K, 1])
        end_sb = load_w("end", endv[:, :], [K, 1])

        ones16 = wp.tile([1, BL], BF16, tag="ones16")
        nc.vector.memset(ones16[:], 1.0)
        ones2020 = wp.tile([K, K], F32, tag="ones2020")
        nc.vector.memset(ones2020[:], 1.0)
        eexp = wp.tile([K, K], F32, tag="eexp")
        nc.scalar.activation(eexp[:], trans_sb[:], AF.Exp)
        eexpT = wp.tile([K, K], F32, tag="eexpT")
        nc.scalar.activation(eexpT[:], transT_sb[:], AF.Exp)
        expstart = wp.tile([K, 1], F32, tag="expstart")
        nc.scalar.activation(expstart[:], start_sb[:], AF.Exp)
        expend = wp.tile([K, 1], F32, tag="expend")
        nc.scalar.activation(expend[:], end_sb[:], AF.Exp)

        # Embedding gather (+transpose): xg[128_E, NTB] bf16, col = t*BL+b.
        # Chunks ordered head/tail interleaved so both scan directions can
        # start as soon as their end of the sequence has landed.
        xg = big.tile([128, 1, NTB], BF16, tag="bigX")
        GCH = 256  # idxs per gather (SWDGE descriptor-ring limit)
        ngch = max(1, NTB // GCH)
        gorder = []
        for i in range((ngch + 1) // 2):
            gorder.append(i)
            if ngch - 1 - i != i:
                gorder.append(ngch - 1 - i)
        for g in gorder:
            cw = min(GCH, NTB)
            nc.gpsimd.dma_gather(
                xg[:, :, g * cw:(g + 1) * cw], embedb[:, :],
                idx[:, g * (cw // 16):(g + 1) * (cw // 16)],
                cw, cw, E, transpose=True)

        # Histories (feature-on-partition, col = t*BL + b), bf16
        h0T = [big.tile([H, NTB], BF16, tag=f"h0T{d}", name=f"h0T{d}")
               for d in range(2)]
        h1T = [big.tile([H, NTB], BF16, tag=f"h1T{d}", name=f"h1T{d}")
               for d in range(2)]

        # Emissions are produced chunk-by-chunk inside the layer-1 scan, as
        # soon as both directions have crossed a chunk's tick range.
        emr = big.tile([K, NTB], BF16, tag="emr")     # b-major: col=b*nt+t
        expem = big.tile([K, NTB], F32, tag="expem")  # t-major: col=t*BL+b
        ECH = 512 if NTB % 512 == 0 else NTB
        etch = ECH // BL                              # t per chunk
        nech = NTB // ECH
        # all chunks are emitted after the last scan tick, outside-in: the
        # CRF alpha/beta chains need chunks 0 and nech-1 first, then chase
        # the remaining emissions (emission rate >> CRF consumption rate).
        corder = []
        for i in range((nech + 1) // 2):
            corder.append(i)
            if nech - 1 - i != i:
                corder.append(nech - 1 - i)
        em_ready = {nt - 1: corder}

        def emit_emission_chunk(ep, c):
            pe = ep.tile([K, ECH], F32)
            sl = slice(c * ECH, (c + 1) * ECH)
            _mm(nc, pe[:], wout_sb[0][:], h1T[0][:, sl], True, False)
            _mm(nc, pe[:], wout_sb[1][:], h1T[1][:, sl], False, True)
            hh_n = 1
            hw = etch // hh_n
            for h in range(hh_n):
                tsl0 = h * hw
                # write em (+bout) b-major via strided AP
                pe3 = pe.rearrange("p (t b) -> p t b", b=BL)[
                    :, tsl0:tsl0 + hw, :]
                emr3 = emr.rearrange("p (b t) -> p b t", b=BL)[
                    :, :, c * etch + tsl0:c * etch + tsl0 + hw
                    ].rearrange("p b t -> p t b")
                nc.scalar.activation(emr3, pe3, AF.Identity, bias=bout_sb[:])
                # exp(em + bout) t-major, straight from psum
                csl = slice(c * ECH + tsl0 * BL, c * ECH + (tsl0 + hw) * BL)
                nc.scalar.activation(expem[:, csl],
                                     pe[:, tsl0 * BL:(tsl0 + hw) * BL],
                                     AF.Exp, bias=bout_sb[:])

        # ---------------- P1 / P2: the two BiLSTM layers ----------------
        def scan_layer(layer, hist_out):
            """One BiLSTM layer: fwd+bwd scans as two independent chains.

            All per-tick tiles are [feat(128), batch(BL)]; the four gate
            blocks sit side by side in a [128, 4*BL] psum tile.
            """
            whh = whh0_sb if layer == 0 else whh1_sb
            bb = b0_sb if layer == 0 else b1_sb
            with tc.tile_pool(name=f"zp{layer}", bufs=2, space="PSUM") as zp, \
                 tc.tile_pool(name=f"ep{layer}", bufs=2,
                              space="PSUM") as ep:
                cprev = [None, None]
                zs = [None, None]
                for n in range(nt):
                    tt = [n, nt - 1 - n]     # [fwd t, bwd t]
                    # --- PE: bias + x-proj for both dirs, then h-proj ---
                    for d in range(2):
                        t_ = tt[d]
                        sl = slice(t_ * BL, (t_ + 1) * BL)
                        z = zp.tile([H, 4 * BL], F32, tag=f"z{d}",
                                    name=f"z{d}")
                        zs[d] = z
                        for blk in range(4):
                            zb = z[:, blk * BL:(blk + 1) * BL]
                            bs = slice(blk * H, (blk + 1) * H)
                            _mm(nc, zb, bb[d][0:1, bs], ones16[:],
                                start=True, stop=False)
                            if layer == 0:
                                _mm(nc, zb, wih0_sb[d][:, bs],
                                    xg[:, 0, sl], start=False, stop=(n == 0))
                            else:
                                _mm(nc, zb, wih1_sb[d][0][:, bs],
                                    h0T[0][:, sl], start=False, stop=False)
                                _mm(nc, zb, wih1_sb[d][1][:, bs],
                                    h0T[1][:, sl], start=False, stop=(n == 0))
                    for d in range(2):
                        if n == 0:
                            continue
                        t_ = tt[d]
                        tprev = t_ + (-1 if d == 0 else 1)
                        psl = slice(tprev * BL, (tprev + 1) * BL)
                        z = zs[d]
                        for blk in range(4):
                            zb = z[:, blk * BL:(blk + 1) * BL]
                            bs = slice(blk * H, (blk + 1) * H)
                            _mm(nc, zb, whh[d][:, bs], hist_out[d][:, psl],
                                start=False, stop=True)
                    # --- Act: the one big sigmoid per dir ---
                    ss = []
                    for d in range(2):
                        s = work.tile([H, 4 * BL], F32, tag=f"s{d}",
                                      name=f"s{d}")
                        nc.scalar.activation(s[:], zs[d][:], AF.Sigmoid)
                        ss.append(s)
                    # --- DVE: cell update per dir ---
                    cns = []
                    for d in range(2):
                        s = ss[d]
                        si = s[:, 0 * BL:1 * BL]
                        sf = s[:, 1 * BL:2 * BL]
                        sg = s[:, 2 * BL:3 * BL]
                        u = work.tile([H, BL], F32, tag=f"u{d}", name=f"u{d}")
                        nc.vector.scalar_tensor_tensor(
                            u[:], sg, -0.5, si, OP.add, OP.mult)
                        if n == 0:
                            cns.append(u)
                            cprev[d] = u
                            continue
                        fc = work.tile([H, BL], F32, tag=f"fc{d}",
                                       name=f"fc{d}")
                        nc.vector.tensor_tensor(fc[:], sf, cprev[d][:],
                                                OP.mult)
                        cnew = stp.tile([H, BL], F32, tag=f"c{layer}{d}",
                                        name=f"cn{layer}{d}")
                        nc.vector.tensor_tensor(cnew[:], fc[:], u[:], OP.add)
                        cns.append(cnew)
                        cprev[d] = cnew
                    # --- Act: c-path sigmoid; DVE: h into history ---
                    scs = []
                    for d in range(2):
                        sc = work.tile([H, BL], F32, tag=f"sc{d}",
                                       name=f"sc{d}")
                        nc.scalar.activation(sc[:], cns[d][:], AF.Sigmoid,
                                             scale=4.0)
                        scs.append(sc)
                    for d in range(2):
                        t_ = tt[d]
                        sl = slice(t_ * BL, (t_ + 1) * BL)
                        so = ss[d][:, 3 * BL:4 * BL]
                        nc.vector.scalar_tensor_tensor(
                            hist_out[d][:, sl], scs[d][:], -0.5, so,
                            OP.add, OP.mult)
                    if layer == 1 and n in em_ready:
                        for c in em_ready[n]:
                            emit_emission_chunk(ep, c)

        scan_layer(0, h0T)
        scan_layer(1, h1T)

        # ------- P3b: CRF partition function (bidirectional) -------
        MID = nt // 2   # alpha covers t=0..MID-1, beta covers t=MID..nt-1
        with tc.tile_pool(name="cp", bufs=1, space="PSUM") as cp, \
             tc.tile_pool(name="sp", bufs=1, space="PSUM") as sp, \
             tc.tile_pool(name="nwork", bufs=2) as nwork:
            # chain 0: alpha from t=0; chain 1: beta from t=nt-1
            aps, logaccs, pendings = [], [], []
            for hh in range(2):
                t0 = 0 if hh == 0 else nt - 1
                sl0 = slice(t0 * BL, (t0 + 1) * BL)
                a0 = stp.tile([K, BL], F32, tag=f"alpha{hh}", name=f"a0_{hh}")
                ini = expstart if hh == 0 else expend
                nc.vector.tensor_tensor(
                    a0[:], expem[:, sl0],
                    ini[:, 0:1].to_broadcast([K, BL]), OP.mult)
                la0 = stp.tile([1, BL], F32, tag=f"logacc{hh}",
                               name=f"la0_{hh}")
                nc.vector.memset(la0[:], 0.0)
                aps.append(a0)
                logaccs.append(la0)
                pendings.append(None)
            nsteps = [MID - 1, nt - 1 - MID]   # alpha: 1..MID-1; beta: nt-2..MID
            emat = [eexp, eexpT]

            # --- numerator: device computes only sum_t em[y_t, t] per b;
            # the tags-only part (trans/start/end sums) is added on the host.
            tagsb = big.tile([K, NTB], BF16, tag="tags_rep", name="tagsb")
            nc.sync.dma_start(out=tagsb[:], in_=tagoh[:, :])
            scol = stp.tile([K, BL], F32, tag="scol")

            def num_batch(b):
                base = b * nt
                dump = nwork.tile([K, nt], F32, tag="dump")
                nc.vector.scalar_tensor_tensor(
                    dump[:], emr[:, base:base + nt], 0.0,
                    tagsb[:, base:base + nt],
                    OP.add, OP.mult, accum_out=scol[:, b:b + 1])

            nbq = list(range(BL))  # numerator batches to interleave

            for step in range(1, max(nsteps) + 1):
                for hh in range(2):
                    if step > nsteps[hh]:
                        continue
                    t_ = step if hh == 0 else nt - 1 - step
                    sl = slice(t_ * BL, (t_ + 1) * BL)
                    pa = cp.tile([K, BL], F32, tag=f"pa{hh}", name=f"pa{hh}")
                    _mm(nc, pa[:], emat[hh][:], aps[hh][:], True, True)
                    an = stp.tile([K, BL], F32, tag=f"alpha{hh}",
                                  name=f"an{hh}")
                    nc.vector.tensor_tensor(an[:], pa[:], expem[:, sl],
                                            OP.mult)
                    aps[hh] = an
                    if pendings[hh] is not None and step >= pendings[hh][1]:
                        asc = stp.tile([K, BL], F32, tag=f"alpha{hh}",
                                       name=f"as{hh}")
                        nc.vector.tensor_tensor(
                            asc[:], aps[hh][:], pendings[hh][0][:], OP.mult)
                        aps[hh] = asc
                        pendings[hh] = None
                    if step % RESCALE == 0 and step + 2 < nsteps[hh]:
                        ps = sp.tile([K, BL], F32, tag=f"ps{hh}",
                                     name=f"ps{hh}")
                        _mm(nc, ps[:], ones2020[:], aps[hh][:], True, True)
                        sinv = work.tile([K, BL], F32, tag=f"sinv{hh}",
                                         name=f"sinv{hh}")
                        nc.vector.reciprocal(sinv[:], ps[:])
                        lt = work.tile([1, BL], F32, tag=f"lt{hh}",
                                       name=f"lt{hh}")
                        nc.scalar.activation(lt[:], ps[0:1, :], AF.Ln)
                        la = stp.tile([1, BL], F32, tag=f"logacc{hh}",
                                      name=f"lan{hh}")
                        nc.vector.tensor_tensor(la[:], logaccs[hh][:], lt[:],
                                                OP.add)
                        logaccs[hh] = la
                        pendings[hh] = (sinv, step + 2)
                # interleave one numerator batch every 16 steps
                if step % 16 == 8 and nbq:
                    num_batch(nbq.pop(0))
            while nbq:
                num_batch(nbq.pop(0))

            for hh in range(2):
                if pendings[hh] is not None:
                    asc = stp.tile([K, BL], F32, tag=f"alpha{hh}",
                                   name=f"af{hh}")
                    nc.vector.tensor_tensor(asc[:], aps[hh][:],
                                            pendings[hh][0][:], OP.mult)
                    aps[hh] = asc
            # bridge: Z = (Eexp^T a_{MID-1}) . b_MID  (columnwise dot)
            pa = cp.tile([K, BL], F32, tag="pa0", name="pa_br")
            _mm(nc, pa[:], eexp[:], aps[0][:], True, True)
            w = work.tile([K, BL], F32, tag="wbr")
            nc.vector.tensor_tensor(w[:], pa[:], aps[1][:], OP.mult)
            psf = sp.tile([K, BL], F32, tag="ps0", name="psf")
            _mm(nc, psf[:], ones2020[:], w[:], True, True)
            lnf = work.tile([1, BL], F32, tag="lnf")
            nc.scalar.activation(lnf[:], psf[0:1, :], AF.Ln)
            logz = work.tile([1, BL], F32, tag="logz")
            nc.vector.tensor_tensor(logz[:], lnf[:], logaccs[0][:], OP.add)
            logz2 = work.tile([1, BL], F32, tag="logz2")
            nc.vector.tensor_tensor(logz2[:], logz[:], logaccs[1][:], OP.add)
            nc.sync.dma_start(out=outm[1:2, :], in_=logz2[:])

            # ---------------- P3c: numerator reduction ----------------
            psc = sp.tile([K, BL], F32, tag="ps1", name="psc")
            _mm(nc, psc[:], ones2020[:], scol[:], True, True)
            score = work.tile([1, BL], F32, tag="score")
            nc.vector.tensor_copy(score[:], psc[0:1, :])
            nc.sync.dma_start(out=outm[0:1, :], in_=score[:])

    nc.compile()
    return nc


# ---------------------------------------------------------------------------
# Host side
# ---------------------------------------------------------------------------
_CACHE = {}


def _get_nc(nt):
    if nt not in _CACHE:
        _CACHE[nt] = build(nt)
    return _CACHE[nt]


def prep_inputs(sentences, tags, embed, Wih0, Whh0, b0, Wih1, Whh1, b1,
                Wout, bout, trans, start, end, nt=T):
    """Host-side marshalling: weight transposes + power-of-2 gate rescales."""
    f32 = np.float32
    bf16 = ml_dtypes.bfloat16
    sc = np.ones((G4, 1), f32)
    sc[2 * H:3 * H] = 2.0           # g rows: tanh-via-sigmoid needs 2x

    def stack2(w, s):
        return np.stack([np.ascontiguousarray((w[d] * s).T.astype(bf16))
                         for d in range(2)])

    wihT0 = stack2(Wih0, sc)                    # [2,128,512] (transposed)
    whhT0 = stack2(Whh0, 2.0 * sc)
    b0v = np.stack([(b0[d] * sc[:, 0]).reshape(1, G4).astype(bf16)
                    for d in range(2)])
    wih1T_full = stack2(Wih1, 2.0 * sc)         # [2,256,512]
    wih1T = wih1T_full.reshape(2, 2, H, G4)
    whh1T = stack2(Whh1, 2.0 * sc)
    b1v = np.stack([(b1[d] * sc[:, 0]).reshape(1, G4).astype(bf16)
                    for d in range(2)])
    woutT = np.stack([np.ascontiguousarray((2.0 * Wout[:, :H]).T.astype(bf16)),
                      np.ascontiguousarray((2.0 * Wout[:, H:]).T.astype(bf16))])
    shared = dict(
        embedb=np.ascontiguousarray(embed.astype(bf16)),
        wihT0=wihT0, whhT0=whhT0, b0v=b0v, wih1T=wih1T, whh1T=whh1T, b1v=b1v,
        woutT=woutT, boutv=bout.reshape(K, 1).astype(f32),
        transm=trans.astype(f32),
        transmT=np.ascontiguousarray(trans.T.astype(f32)),
        startv=start.reshape(K, 1).astype(f32),
        endv=end.reshape(K, 1).astype(f32),
    )
    in_maps = []
    for c in range(NCORES):
        bsl = slice(c * BL, (c + 1) * BL)
        m = dict(shared)
        m["toks16"] = np.ascontiguousarray(
            sentences[bsl, :nt].astype(np.int16))
        toh = (tags[bsl, :nt][:, None, :] ==
               np.arange(K)[None, :, None])          # [BL, K, nt]
        m["tagoh"] = np.ascontiguousarray(
            toh.transpose(1, 0, 2).reshape(K, BL * nt).astype(bf16))
        in_maps.append(m)
    return in_maps


def run(inputs_np, nt=T, trace=False):
    nc = _get_nc(nt)
    in_maps = prep_inputs(
        inputs_np["sentences"], inputs_np["tags"], inputs_np["embed"],
        inputs_np["Wih0"], inputs_np["Whh0"], inputs_np["b0"],
        inputs_np["Wih1"], inputs_np["Whh1"], inputs_np["b1"],
        inputs_np["Wout"], inputs_np["bout"], inputs_np["trans"],
        inputs_np["start"], inputs_np["end"], nt=nt)
    res = run_bass_kernel_spmd(nc, in_maps, core_ids=list(range(NCORES)),
                               trace=trace)
    score = np.concatenate([res.results[c]["outm"][0] for c in range(NCORES)])
    logz = np.concatenate([res.results[c]["outm"][1] for c in range(NCORES)])
    # tags-only numerator part (start/end/transition sums) on the host
    y = np.asarray(inputs_np["tags"])[:, :nt].astype(np.int64)
    trans = np.asarray(inputs_np["trans"], dtype=np.float64)
    sh = (np.asarray(inputs_np["start"], dtype=np.float64)[y[:, 0]]
          + np.asarray(inputs_np["end"], dtype=np.float64)[y[:, -1]]
          + trans[y[:, :-1], y[:, 1:]].sum(axis=1))
    loss = -np.mean(score + sh - logz)
    return np.float32(loss), res


def kernel(**inputs):
    inputs_np = {k: np.asarray(v) for k, v in inputs.items()}
    loss, _ = run(inputs_np, nt=T)
    return np.asarray(loss, dtype=np.float32)


# revision 17
# speedup vs baseline: 1.0020x; 1.0020x over previous
"""BiLSTM-CRF loss kernel for 8x Trainium2 NeuronCores (Bass/Tile).

Sharding: data-parallel over batch (16 sentences per core). Each core runs the
identical SPMD program: embedding gather -> 2 BiLSTM layers (fwd+bwd scans
interleaved per tick) -> emissions -> bidirectional CRF partition function +
path-score numerator. Host sums per-core partials and adds the tags-only part
of the numerator.

Design notes (v3, latency-oriented):
 - The per-tick recurrence is latency-bound (engines are mostly idle), so the
   cell is laid out feature-on-partition: z/s/c/h are [128, batch] tiles.
   Act/DVE instruction cost = free-dim elems + fixed access-latency init, so
   narrow free dims win, and h is produced in exactly the [H, B] layout the
   next tick's matmul consumes (no transpose, no history copy).
 - Gate preactivations per direction: psum tile [128, 4*BL]; per gate block a
   rank-1 bias matmul + x-proj + h-proj accumulate in bf16 (1 PE cycle/row).
 - tanh(x) = 2*sigmoid(2x) - 1 everywhere, so one Sigmoid activation covers
   all four gates.  With h~ = h/2 and c~ = c/2 (factors of 2 folded into the
   weights host-side):
     s = sigmoid(z'), z' row-scaled so s_g = sigmoid(2 z_g)
     u  = (s_g - 0.5) * s_i          ( = i*g/2 )
     c~ = s_f * c~_prev + u
     h~ = (sigmoid(4 c~) - 0.5) * s_o
 - Per-tick critical cycle: h-matmul -> sigmoid[128,64] -> u/fc/cn (DVE) ->
   sigmoid(4c)[128,16] -> h-write (DVE, straight into the history buffer).
   fwd and bwd run as two independent chains that fill each other's idle.
 - Emissions are computed after the scans, outside-in (chunks 0 and N-1
   first) so the CRF can start immediately and the remaining emission chunks
   hide under its latency-bound chains.
 - CRF partition function is bidirectional to halve the sequential chain:
   alpha runs t=0..M-1 (exp space, a_t = e_t .* (Eexp^T a_{t-1})), beta runs
   t=T-1..M (b_t = e_t .* (Eexp b_{t+1}), includes end), concurrently; then
   Z = (Eexp^T a_{M-1}) . b_M.  Partition-sum rescale every RESCALE steps
   (applied two steps deferred, off the critical chain), log accumulated.
 - Numerator: device computes only sum_t em[y_t, t] per sentence (one
   accumulating STT against host-marshalled one-hot tags, interleaved into
   the CRF); the tags-only part (start/end/transition sums) is computed on
   the host directly from the inputs.
"""

import sys

sys.path.insert(0, "/opt/trn_rl_repo")

import contextlib

import numpy as np
import ml_dtypes

import concourse.bass as bass
import concourse.tile as tile
from concourse import bacc, mybir
from concourse.bass_utils import run_bass_kernel_spmd

F32 = mybir.dt.float32
F32R = mybir.dt.float32r
BF16 = mybir.dt.bfloat16
I16 = mybir.dt.int16
AF = mybir.ActivationFunctionType
OP = mybir.AluOpType

NCORES = 8
B, T, E, H, K, V = 128, 512, 128, 128, 20, 30000
G4 = 4 * H          # 512
BL = B // NCORES    # 16 sentences per core
RESCALE = 8


def _mm(nc, out, lhsT, rhs, start, stop):
    nc.tensor.matmul(out, lhsT, rhs, start=start, stop=stop)


def build(nt=T):
    """Build the SPMD program for sequence length nt (nt=T for real use)."""
    nc = bacc.Bacc("TRN2", target_bir_lowering=False, debug=False,
                   num_devices=NCORES)
    NTB = nt * BL   # flattened (t,b) count per core

    # ---- DRAM I/O ----
    embedb = nc.dram_tensor("embedb", [V, E], BF16, kind="ExternalInput")
    toks16 = nc.dram_tensor("toks16", [BL, nt], I16, kind="ExternalInput")
    tagoh = nc.dram_tensor("tagoh", [K, NTB], BF16, kind="ExternalInput")  # one-hot, b-major
    wihT0 = nc.dram_tensor("wihT0", [2, E, G4], BF16, kind="ExternalInput")
    whhT0 = nc.dram_tensor("whhT0", [2, H, G4], BF16, kind="ExternalInput")
    b0v = nc.dram_tensor("b0v", [2, 1, G4], BF16, kind="ExternalInput")
    wih1T = nc.dram_tensor("wih1T", [2, 2, H, G4], BF16, kind="ExternalInput")
    whh1T = nc.dram_tensor("whh1T", [2, H, G4], BF16, kind="ExternalInput")
    b1v = nc.dram_tensor("b1v", [2, 1, G4], BF16, kind="ExternalInput")
    woutT = nc.dram_tensor("woutT", [2, H, K], BF16, kind="ExternalInput")
    boutv = nc.dram_tensor("boutv", [K, 1], F32, kind="ExternalInput")
    transm = nc.dram_tensor("transm", [K, K], F32, kind="ExternalInput")
    transmT = nc.dram_tensor("transmT", [K, K], F32, kind="ExternalInput")
    startv = nc.dram_tensor("startv", [K, 1], F32, kind="ExternalInput")
    endv = nc.dram_tensor("endv", [K, 1], F32, kind="ExternalInput")
    outm = nc.dram_tensor("outm", [2, BL], F32, kind="ExternalOutput")

    with tile.TileContext(nc) as tc, contextlib.ExitStack() as ctx:
        big = ctx.enter_context(tc.tile_pool(name="big", bufs=1))
        wp = ctx.enter_context(tc.tile_pool(name="wp", bufs=1))
        work = ctx.enter_context(tc.tile_pool(name="work", bufs=3))
        stp = ctx.enter_context(tc.tile_pool(name="stp", bufs=2))

        # ---------------- P0: constants, weights, gather ----------------
        idx = wp.tile([128, nt], I16, tag="idx")
        nc.gpsimd.memset(idx[:], 0)
        nc.sync.dma_start(out=idx[0:BL, :], in_=toks16[:, :])

        def load_w(name, dram_ap, shape, dt=F32):
            t = wp.tile(shape, dt, tag=name)
            nc.sync.dma_start(out=t[:], in_=dram_ap)
            return t

        wih0_sb = [load_w(f"wih0_{d}", wihT0[d], [E, G4], BF16)
                   for d in range(2)]
        whh0_sb = [load_w(f"whh0_{d}", whhT0[d], [H, G4], BF16)
                   for d in range(2)]
        b0_sb = [load_w(f"b0_{d}", b0v[d], [1, G4], BF16) for d in range(2)]
        wih1_sb = [[load_w(f"wih1_{d}{h}", wih1T[d, h], [H, G4], BF16)
                    for h in range(2)] for d in range(2)]
        whh1_sb = [load_w(f"whh1_{d}", whh1T[d], [H, G4], BF16)
                   for d in range(2)]
        b1_sb = [load_w(f"b1_{d}", b1v[d], [1, G4], BF16) for d in range(2)]
        wout_sb = [load_w(f"wout_{d}", woutT[d], [H, K], BF16)
                   for d in range(2)]
        bout_sb = load_w("bout", boutv[:, :], [K, 1])
        trans_sb = load_w("trans", transm[:, :], [K, K])
        transT_sb = load_w("transT", transmT[:, :], [K, K])
        start_sb = load_w("start", startv[:, :], [K, 1])
        end_sb = load_w("end", endv[:, :], [K, 1])

        ones16 = wp.tile([1, BL], BF16, tag="ones16")
        nc.vector.memset(ones16[:], 1.0)
        ones2020 = wp.tile([K, K], F32, tag="ones2020")
        nc.vector.memset(ones2020[:], 1.0)
        eexp = wp.tile([K, K], F32, tag="eexp")
        nc.scalar.activation(eexp[:], trans_sb[:], AF.Exp)
        eexpT = wp.tile([K, K], F32, tag="eexpT")
        nc.scalar.activation(eexpT[:], transT_sb[:], AF.Exp)
        expstart = wp.tile([K, 1], F32, tag="expstart")
        nc.scalar.activation(expstart[:], start_sb[:], AF.Exp)
        expend = wp.tile([K, 1], F32, tag="expend")
        nc.scalar.activation(expend[:], end_sb[:], AF.Exp)

        # Embedding gather (+transpose): xg[128_E, NTB] bf16, col = t*BL+b.
        # Chunks ordered head/tail interleaved so both scan directions can
        # start as soon as their end of the sequence has landed.
        xg = big.tile([128, 1, NTB], BF16, tag="bigX")
        GCH = 256  # idxs per gather (SWDGE descriptor-ring limit)
        ngch = max(1, NTB // GCH)
        gorder = []
        for i in range((ngch + 1) // 2):
            gorder.append(i)
            if ngch - 1 - i != i:
                gorder.append(ngch - 1 - i)
        for g in gorder:
            cw = min(GCH, NTB)
            nc.gpsimd.dma_gather(
                xg[:, :, g * cw:(g + 1) * cw], embedb[:, :],
                idx[:, g * (cw // 16):(g + 1) * (cw // 16)],
                cw, cw, E, transpose=True)

        # Histories (feature-on-partition, col = t*BL + b), bf16
        h0T = [big.tile([H, NTB], BF16, tag=f"h0T{d}", name=f"h0T{d}")
               for d in range(2)]
        h1T = [big.tile([H, NTB], BF16, tag=f"h1T{d}", name=f"h1T{d}")
               for d in range(2)]

        # Emissions are produced chunk-by-chunk inside the layer-1 scan, as
        # soon as both directions have crossed a chunk's tick range.
        emr = big.tile([K, NTB], BF16, tag="emr")     # b-major: col=b*nt+t
        expem = big.tile([K, NTB], F32, tag="expem")  # t-major: col=t*BL+b
        ECH = 512 if NTB % 512 == 0 else NTB
        etch = ECH // BL                              # t per chunk
        nech = NTB // ECH
        # all chunks are emitted after the last scan tick, outside-in: the
        # CRF alpha/beta chains need chunks 0 and nech-1 first, then chase
        # the remaining emissions (emission rate >> CRF consumption rate).
        corder = []
        for i in range((nech + 1) // 2):
            corder.append(i)
            if nech - 1 - i != i:
                corder.append(nech - 1 - i)
        em_ready = {nt - 1: corder[:2]}
        em_defer = corder[2:]

        def emit_emission_chunk(ep, c):
            pe = ep.tile([K, ECH], F32)
            sl = slice(c * ECH, (c + 1) * ECH)
            _mm(nc, pe[:], wout_sb[0][:], h1T[0][:, sl], True, False)
            _mm(nc, pe[:], wout_sb[1][:], h1T[1][:, sl], False, True)
            hh_n = 1
            hw = etch // hh_n
            for h in range(hh_n):
                tsl0 = h * hw
                # write em (+bout) b-major via strided AP
                pe3 = pe.rearrange("p (t b) -> p t b", b=BL)[
                    :, tsl0:tsl0 + hw, :]
                emr3 = emr.rearrange("p (b t) -> p b t", b=BL)[
                    :, :, c * etch + tsl0:c * etch + tsl0 + hw
                    ].rearrange("p b t -> p t b")
                nc.scalar.activation(emr3, pe3, AF.Identity, bias=bout_sb[:])
                # exp(em + bout) t-major, straight from psum
                csl = slice(c * ECH + tsl0 * BL, c * ECH + (tsl0 + hw) * BL)
                nc.scalar.activation(expem[:, csl],
                                     pe[:, tsl0 * BL:(tsl0 + hw) * BL],
                                     AF.Exp, bias=bout_sb[:])

        # ---------------- P1 / P2: the two BiLSTM layers ----------------
        def scan_layer(layer, hist_out):
            """One BiLSTM layer: fwd+bwd scans as two independent chains.

            All per-tick tiles are [feat(128), batch(BL)]; the four gate
            blocks sit side by side in a [128, 4*BL] psum tile.
            """
            whh = whh0_sb if layer == 0 else whh1_sb
            bb = b0_sb if layer == 0 else b1_sb
            with tc.tile_pool(name=f"zp{layer}", bufs=2, space="PSUM") as zp, \
                 tc.tile_pool(name=f"ep{layer}", bufs=2,
                              space="PSUM") as ep:
                cprev = [None, None]
                zs = [None, None]
                for n in range(nt):
                    tt = [n, nt - 1 - n]     # [fwd t, bwd t]
                    # --- PE: bias + x-proj for both dirs, then h-proj ---
                    for d in range(2):
                        t_ = tt[d]
                        sl = slice(t_ * BL, (t_ + 1) * BL)
                        z = zp.tile([H, 4 * BL], F32, tag=f"z{d}",
                                    name=f"z{d}")
                        zs[d] = z
                        for blk in range(4):
                            zb = z[:, blk * BL:(blk + 1) * BL]
                            bs = slice(blk * H, (blk + 1) * H)
                            _mm(nc, zb, bb[d][0:1, bs], ones16[:],
                                start=True, stop=False)
                            if layer == 0:
                                _mm(nc, zb, wih0_sb[d][:, bs],
                                    xg[:, 0, sl], start=False, stop=(n == 0))
                            else:
                                _mm(nc, zb, wih1_sb[d][0][:, bs],
                                    h0T[0][:, sl], start=False, stop=False)
                                _mm(nc, zb, wih1_sb[d][1][:, bs],
                                    h0T[1][:, sl], start=False, stop=(n == 0))
                    for d in range(2):
                        if n == 0:
                            continue
                        t_ = tt[d]
                        tprev = t_ + (-1 if d == 0 else 1)
                        psl = slice(tprev * BL, (tprev + 1) * BL)
                        z = zs[d]
                        for blk in range(4):
                            zb = z[:, blk * BL:(blk + 1) * BL]
                            bs = slice(blk * H, (blk + 1) * H)
                            _mm(nc, zb, whh[d][:, bs], hist_out[d][:, psl],
                                start=False, stop=True)
                    # --- Act: the one big sigmoid per dir ---
                    ss = []
                    for d in range(2):
                        s = work.tile([H, 4 * BL], F32, tag=f"s{d}",
                                      name=f"s{d}")
                        nc.scalar.activation(s[:], zs[d][:], AF.Sigmoid)
                        ss.append(s)
                    # --- DVE: cell update per dir ---
                    cns = []
                    for d in range(2):
                        s = ss[d]
                        si = s[:, 0 * BL:1 * BL]
                        sf = s[:, 1 * BL:2 * BL]
                        sg = s[:, 2 * BL:3 * BL]
                        u = work.tile([H, BL], F32, tag=f"u{d}", name=f"u{d}")
                        nc.vector.scalar_tensor_tensor(
                            u[:], sg, -0.5, si, OP.add, OP.mult)
                        if n == 0:
                            cns.append(u)
                            cprev[d] = u
                            continue
                        fc = work.tile([H, BL], F32, tag=f"fc{d}",
                                       name=f"fc{d}")
                        nc.vector.tensor_tensor(fc[:], sf, cprev[d][:],
                                                OP.mult)
                        cnew = stp.tile([H, BL], F32, tag=f"c{layer}{d}",
                                        name=f"cn{layer}{d}")
                        nc.vector.tensor_tensor(cnew[:], fc[:], u[:], OP.add)
                        cns.append(cnew)
                        cprev[d] = cnew
                    # --- Act: c-path sigmoid; DVE: h into history ---
                    scs = []
                    for d in range(2):
                        sc = work.tile([H, BL], F32, tag=f"sc{d}",
                                       name=f"sc{d}")
                        nc.scalar.activation(sc[:], cns[d][:], AF.Sigmoid,
                                             scale=4.0)
                        scs.append(sc)
                    for d in range(2):
                        t_ = tt[d]
                        sl = slice(t_ * BL, (t_ + 1) * BL)
                        so = ss[d][:, 3 * BL:4 * BL]
                        nc.vector.scalar_tensor_tensor(
                            hist_out[d][:, sl], scs[d][:], -0.5, so,
                            OP.add, OP.mult)
                    if layer == 1 and n in em_ready:
                        for c in em_ready[n]:
                            emit_emission_chunk(ep, c)

        scan_layer(0, h0T)
        scan_layer(1, h1T)

        # ------- P3b: CRF partition function (bidirectional) -------
        MID = nt // 2   # alpha covers t=0..MID-1, beta covers t=MID..nt-1
        with tc.tile_pool(name="cp", bufs=1, space="PSUM") as cp, \
             tc.tile_pool(name="sp", bufs=1, space="PSUM") as sp, \
             tc.tile_pool(name="ep2", bufs=2, space="PSUM") as ep2, \
             tc.tile_pool(name="nwork", bufs=2) as nwork:
            # chain 0: alpha from t=0; chain 1: beta from t=nt-1
            aps, logaccs, pendings = [], [], []
            for hh in range(2):
                t0 = 0 if hh == 0 else nt - 1
                sl0 = slice(t0 * BL, (t0 + 1) * BL)
                a0 = stp.tile([K, BL], F32, tag=f"alpha{hh}", name=f"a0_{hh}")
                ini = expstart if hh == 0 else expend
                nc.vector.tensor_tensor(
                    a0[:], expem[:, sl0],
                    ini[:, 0:1].to_broadcast([K, BL]), OP.mult)
                la0 = stp.tile([1, BL], F32, tag=f"logacc{hh}",
                               name=f"la0_{hh}")
                nc.vector.memset(la0[:], 0.0)
                aps.append(a0)
                logaccs.append(la0)
                pendings.append(None)
            nsteps = [MID - 1, nt - 1 - MID]   # alpha: 1..MID-1; beta: nt-2..MID
            emat = [eexp, eexpT]

            # --- numerator: device computes only sum_t em[y_t, t] per b;
            # the tags-only part (trans/start/end sums) is added on the host.
            tagsb = big.tile([K, NTB], BF16, tag="tags_rep", name="tagsb")
            nc.sync.dma_start(out=tagsb[:], in_=tagoh[:, :])
            scol = stp.tile([K, BL], F32, tag="scol")

            def num_batch(b):
                base = b * nt
                dump = nwork.tile([K, nt], F32, tag="dump")
                nc.vector.scalar_tensor_tensor(
                    dump[:], emr[:, base:base + nt], 0.0,
                    tagsb[:, base:base + nt],
                    OP.add, OP.mult, accum_out=scol[:, b:b + 1])

            nbq = list(range(BL))  # numerator batches to interleave

            for step in range(1, max(nsteps) + 1):
                for hh in range(2):
                    if step > nsteps[hh]:
                        continue
                    t_ = step if hh == 0 else nt - 1 - step
                    sl = slice(t_ * BL, (t_ + 1) * BL)
                    pa = cp.tile([K, BL], F32, tag=f"pa{hh}", name=f"pa{hh}")
                    _mm(nc, pa[:], emat[hh][:], aps[hh][:], True, True)
                    an = stp.tile([K, BL], F32, tag=f"alpha{hh}",
                                  name=f"an{hh}")
                    nc.vector.tensor_tensor(an[:], pa[:], expem[:, sl],
                                            OP.mult)
                    aps[hh] = an
                    if pendings[hh] is not None and step >= pendings[hh][1]:
                        asc = stp.tile([K, BL], F32, tag=f"alpha{hh}",
                                       name=f"as{hh}")
                        nc.vector.tensor_tensor(
                            asc[:], aps[hh][:], pendings[hh][0][:], OP.mult)
                        aps[hh] = asc
                        pendings[hh] = None
                    if step % RESCALE == 0 and step + 2 < nsteps[hh]:
                        ps = sp.tile([K, BL], F32, tag=f"ps{hh}",
                                     name=f"ps{hh}")
                        _mm(nc, ps[:], ones2020[:], aps[hh][:], True, True)
                        sinv = work.tile([K, BL], F32, tag=f"sinv{hh}",
                                         name=f"sinv{hh}")
                        nc.vector.reciprocal(sinv[:], ps[:])
                        lt = work.tile([1, BL], F32, tag=f"lt{hh}",
                                       name=f"lt{hh}")
                        nc.scalar.activation(lt[:], ps[0:1, :], AF.Ln)
                        la = stp.tile([1, BL], F32, tag=f"logacc{hh}",
                                      name=f"lan{hh}")
                        nc.vector.tensor_tensor(la[:], logaccs[hh][:], lt[:],
                                                OP.add)
                        logaccs[hh] = la
                        pendings[hh] = (sinv, step + 2)
                # interleave one numerator batch every 16 steps
                if step % 16 == 8 and nbq:
                    num_batch(nbq.pop(0))
                # interleave deferred emission chunks (ends already done);
                # the chains consume ~16 t per 16 steps, chunks hold 32 t,
                # so emitting one per 8 steps stays well ahead.
                if step % 8 == 4 and em_defer:
                    emit_emission_chunk(ep2, em_defer.pop(0))
            while nbq:
                num_batch(nbq.pop(0))

            for hh in range(2):
                if pendings[hh] is not None:
                    asc = stp.tile([K, BL], F32, tag=f"alpha{hh}",
                                   name=f"af{hh}")
                    nc.vector.tensor_tensor(asc[:], aps[hh][:],
                                            pendings[hh][0][:], OP.mult)
                    aps[hh] = asc
            # bridge: Z = (Eexp^T a_{MID-1}) . b_MID  (columnwise dot)
            pa = cp.tile([K, BL], F32, tag="pa0", name="pa_br")
            _mm(nc, pa[:], eexp[:], aps[0][:], True, True)
            w = work.tile([K, BL], F32, tag="wbr")
            nc.vector.tensor_tensor(w[:], pa[:], aps[1][:], OP.mult)
            psf = sp.tile([K, BL], F32, tag="ps0", name="psf")
            _mm(nc, psf[:], ones2020[:], w[:], True, True)
            lnf = work.tile([1, BL], F32, tag="lnf")
            nc.scalar.activation(lnf[:], psf[0:1, :], AF.Ln)
            logz = work.tile([1, BL], F32, tag="logz")
            nc.vector.tensor_tensor(logz[:], lnf[:], logaccs[0][:], OP.add)
            logz2 = work.tile([1, BL], F32, tag="logz2")
            nc.vector.tensor_tensor(logz2[:], logz[:], logaccs[1][:], OP.add)
            nc.sync.dma_start(out=outm[1:2, :], in_=logz2[:])

            # ---------------- P3c: numerator reduction ----------------
            psc = sp.tile([K, BL], F32, tag="ps1", name="psc")
            _mm(nc, psc[:], ones2020[:], scol[:], True, True)
            score = work.tile([1, BL], F32, tag="score")
            nc.vector.tensor_copy(score[:], psc[0:1, :])
            nc.sync.dma_start(out=outm[0:1, :], in_=score[:])

    nc.compile()
    return nc


# ---------------------------------------------------------------------------
# Host side
# ---------------------------------------------------------------------------
_CACHE = {}


def _get_nc(nt):
    if nt not in _CACHE:
        _CACHE[nt] = build(nt)
    return _CACHE[nt]


def prep_inputs(sentences, tags, embed, Wih0, Whh0, b0, Wih1, Whh1, b1,
                Wout, bout, trans, start, end, nt=T):
    """Host-side marshalling: weight transposes + power-of-2 gate rescales."""
    f32 = np.float32
    bf16 = ml_dtypes.bfloat16
    sc = np.ones((G4, 1), f32)
    sc[2 * H:3 * H] = 2.0           # g rows: tanh-via-sigmoid needs 2x

    def stack2(w, s):
        return np.stack([np.ascontiguousarray((w[d] * s).T.astype(bf16))
                         for d in range(2)])

    wihT0 = stack2(Wih0, sc)                    # [2,128,512] (transposed)
    whhT0 = stack2(Whh0, 2.0 * sc)
    b0v = np.stack([(b0[d] * sc[:, 0]).reshape(1, G4).astype(bf16)
                    for d in range(2)])
    wih1T_full = stack2(Wih1, 2.0 * sc)         # [2,256,512]
    wih1T = wih1T_full.reshape(2, 2, H, G4)
    whh1T = stack2(Whh1, 2.0 * sc)
    b1v = np.stack([(b1[d] * sc[:, 0]).reshape(1, G4).astype(bf16)
                    for d in range(2)])
    woutT = np.stack([np.ascontiguousarray((2.0 * Wout[:, :H]).T.astype(bf16)),
                      np.ascontiguousarray((2.0 * Wout[:, H:]).T.astype(bf16))])
    shared = dict(
        embedb=np.ascontiguousarray(embed.astype(bf16)),
        wihT0=wihT0, whhT0=whhT0, b0v=b0v, wih1T=wih1T, whh1T=whh1T, b1v=b1v,
        woutT=woutT, boutv=bout.reshape(K, 1).astype(f32),
        transm=trans.astype(f32),
        transmT=np.ascontiguousarray(trans.T.astype(f32)),
        startv=start.reshape(K, 1).astype(f32),
        endv=end.reshape(K, 1).astype(f32),
    )
    in_maps = []
    for c in range(NCORES):
        bsl = slice(c * BL, (c + 1) * BL)
        m = dict(shared)
        m["toks16"] = np.ascontiguousarray(
            sentences[bsl, :nt].astype(np.int16))
        toh = (tags[bsl, :nt][:, None, :] ==
               np.arange(K)[None, :, None])          # [BL, K, nt]
        m["tagoh"] = np.ascontiguousarray(
            toh.transpose(1, 0, 2).reshape(K, BL * nt).astype(bf16))
        in_maps.append(m)
    return in_maps


def run(inputs_np, nt=T, trace=False):
    nc = _get_nc(nt)
    in_maps = prep_inputs(
        inputs_np["sentences"], inputs_np["tags"], inputs_np["embed"],
        inputs_np["Wih0"], inputs_np["Whh0"], inputs_np["b0"],
        inputs_np["Wih1"], inputs_np["Whh1"], inputs_np["b1"],
        inputs_np["Wout"], inputs_np["bout"], inputs_np["trans"],
        inputs_np["start"], inputs_np["end"], nt=nt)
    res = run_bass_kernel_spmd(nc, in_maps, core_ids=list(range(NCORES)),
                               trace=trace)
    score = np.concatenate([res.results[c]["outm"][0] for c in range(NCORES)])
    logz = np.concatenate([res.results[c]["outm"][1] for c in range(NCORES)])
    # tags-only numerator part (start/end/transition sums) on the host
    y = np.asarray(inputs_np["tags"])[:, :nt].astype(np.int64)
    trans = np.asarray(inputs_np["trans"], dtype=np.float64)
    sh = (np.asarray(inputs_np["start"], dtype=np.float64)[y[:, 0]]
          + np.asarray(inputs_np["end"], dtype=np.float64)[y[:, -1]]
          + trans[y[:, :-1], y[:, 1:]].sum(axis=1))
    loss = -np.mean(score + sh - logz)
    return np.float32(loss), res


def kernel(**inputs):
    inputs_np = {k: np.asarray(v) for k, v in inputs.items()}
    loss, _ = run(inputs_np, nt=T)
    return np.asarray(loss, dtype=np.float32)
